# revision 1
# baseline (speedup 1.0000x reference)
"""Trainium2 Bass kernel for nn_LstmCloseModel (closed-loop LSTM over basins).

Data-parallel over the ngrid axis: 8 NeuronCores x 375 grid cells each,
replicated weights, full 365-step recurrence on-device per core.

Layout: feature-on-partition, grid-on-free.  State hT/cT live as [128,2,GP]
(H=256 split in two 128-partition chunks); gates are computed transposed
([4H, ngrid]) by PE matmuls with static weight tiles as the stationary
operand, so nothing is transposed inside the time loop.  Matmuls run in
float32r (full-rate PE, needs even dims -> grid padded to GP=376, xcat
K padded to 22).  The Whh matmuls of waves 0-2 are emitted ahead of the
serial pred/fill/x0 chain so the PE stays busy (and HAM-warm) through it,
and the cell tail is split per H-half so next-step k0 matmuls start early.
"""

import os
import sys
import types

sys.path.insert(0, "/opt/trn_rl_repo")

# NTFF profile hook (timing): the image's antenv package lacks axon_hooks;
# inject an equivalent so run_bass_kernel_spmd(trace=True) can measure HW time.
try:
    import antenv

    if not hasattr(antenv, "axon_hooks"):
        from trn_agent_boot.trn_boot import _ntff_profile_via_ctypes

        _hook = _ntff_profile_via_ctypes("/opt/axon/libaxon_pjrt.so")
        _mod = types.ModuleType("antenv.axon_hooks")
        _mod.get_axon_ntff_profile_hook = lambda: _hook
        _mod.set_axon_ntff_profile_hook = lambda h: None
        sys.modules["antenv.axon_hooks"] = _mod
        antenv.axon_hooks = _mod
except Exception:
    pass

import numpy as np

import concourse.bacc as bacc
import concourse.mybir as mybir
import concourse.tile as tile
from concourse.bass import ts
from concourse.bass_utils import run_bass_kernel_spmd

NT, NGRID, NX = 365, 3000, 20
H, NY = 256, 1
NCORES = 8
G = NGRID // NCORES       # 375 grid cells per core
GP = G + (G % 2)          # padded even for float32r matmuls
KXF = NX + 2              # x-feature rows (20 real + 2 zero pads)
KY = 32                   # partition where the y-feedback row lives
F32 = mybir.dt.float32
U8 = mybir.dt.uint8

_dt_env = os.environ.get("LSTM_MM_DT", "f32r")
MM_DT = {"f32r": mybir.dt.float32r, "bf16": mybir.dt.bfloat16, "f32": F32}[_dt_env]

LAST_EXEC_NS = None
LAST_RESULTS = None


def build_nc():
    nc = bacc.Bacc("TRN2")

    xT_d = nc.declare_dram_parameter("xT", [NT, KXF, GP], MM_DT, isOutput=False)
    zz_d = nc.declare_dram_parameter("zz", [1, GP], MM_DT, isOutput=False)
    y0_d = nc.declare_dram_parameter("y0", [NT, GP], F32, isOutput=False)
    mk_d = nc.declare_dram_parameter("mk", [NT, GP], U8, isOutput=False)  # 1=observed
    wih_d = nc.declare_dram_parameter("wihT", [128, 2, 4 * H], MM_DT, isOutput=False)
    whh_d = nc.declare_dram_parameter("whhT", [128, 2, 4 * H], MM_DT, isOutput=False)
    win_d = nc.declare_dram_parameter("winT", [KXF, H], MM_DT, isOutput=False)
    wy2_d = nc.declare_dram_parameter("wy2T", [2, H], MM_DT, isOutput=False)
    wout_d = nc.declare_dram_parameter("woutT", [128, 2], MM_DT, isOutput=False)
    bg_d = nc.declare_dram_parameter("bg", [128, 8], F32, isOutput=False)
    bin_d = nc.declare_dram_parameter("bin", [128, 2], F32, isOutput=False)
    bout_d = nc.declare_dram_parameter("bout", [1, 1], F32, isOutput=False)
    out_d = nc.declare_dram_parameter("outy", [NT, GP], F32, isOutput=True)

    AF = mybir.ActivationFunctionType
    OP = mybir.AluOpType

    with tile.TileContext(nc) as tc:
        with (
            tc.tile_pool(name="singles", bufs=1) as singles,
            tc.tile_pool(name="state", bufs=3) as state,
            tc.tile_pool(name="acts", bufs=3) as acts,
            tc.tile_pool(name="xio", bufs=6) as xio,
            tc.tile_pool(name="ps_x0", bufs=1, space="PSUM") as ps_x0,
            tc.tile_pool(name="ps_g", bufs=6, space="PSUM") as ps_g,
        ):
            # --- constants ---
            wih_s = singles.tile([128, 2, 4 * H], MM_DT)
            whh_s = singles.tile([128, 2, 4 * H], MM_DT)
            win_s = singles.tile([KXF, H], MM_DT)
            wy2_s = singles.tile([KY + 2, H], MM_DT)
            wout_s = singles.tile([128, 2], MM_DT)
            bg_s = singles.tile([128, 8], F32)
            bin_s = singles.tile([128, 2], F32)
            bout_s = singles.tile([1, 1], F32)
            nc.sync.dma_start(out=wih_s[:], in_=wih_d[:])
            nc.sync.dma_start(out=whh_s[:], in_=whh_d[:])
            nc.sync.dma_start(out=win_s[:], in_=win_d[:])
            nc.sync.dma_start(out=wy2_s[KY : KY + 2, :], in_=wy2_d[:])
            nc.sync.dma_start(out=wout_s[:], in_=wout_d[:])
            nc.sync.dma_start(out=bg_s[:], in_=bg_d[:])
            nc.sync.dma_start(out=bin_s[:], in_=bin_d[:])
            nc.sync.dma_start(out=bout_s[:], in_=bout_d[:])

            h_prev = None  # zero at t=0; h/c terms skipped then
            c_prev = None

            for t in range(NT):
                # ---- prediction from h_{t-1}; raw psum also DMA'd out (host
                # adds b_out to the stored output)
                xcat = xio.tile([KY + 2, GP], MM_DT, tag="xcat")
                with nc.named_scope("fill"):
                    nc.sync.dma_start(out=xcat[0:KXF, :], in_=xT_d[t])
                    nc.sync.dma_start(out=xcat[KY + 1 : KY + 2, :], in_=zz_d[:])
                with nc.named_scope("x0"):
                    x0_ps = ps_x0.tile([128, 2, 512], F32, tag="x0ps",
                                       name="x0_ps")
                    for jb in range(2):
                        nc.tensor.matmul(
                            x0_ps[:, jb, 0:GP], win_s[:, ts(jb, 128)],
                            xcat[0:KXF, :], start=True, stop=False,
                        )
                if t > 0:
                    with nc.named_scope("pred"):
                        yo_ps = ps_g.tile([1, GP], F32, tag="gates", name="yo_ps")
                        nc.tensor.matmul(
                            yo_ps[:], wout_s[:, 0:1], h_prev[:, 0, :],
                            start=True, stop=False,
                        )
                        nc.tensor.matmul(
                            yo_ps[:], wout_s[:, 1:2], h_prev[:, 1, :],
                            start=False, stop=True,
                        )

                # ---- Whh matmuls for waves 0..2, k0 chunks then k1 chunks
                g_pss = [
                    [ps_g.tile([128, 512], F32, tag="gates", name=f"gps{w}{jb}")
                     for jb in range(2)]
                    for w in range(3)
                ]
                if t > 0:
                    for k in range(2):
                        for w in range(3):
                            with nc.named_scope(f"whh{w}"):
                                for jb in range(2):
                                    nc.tensor.matmul(
                                        g_pss[w][jb][:, 0:GP],
                                        whh_s[:, k, ts(2 * w + jb, 128)],
                                        h_prev[:, k, :],
                                        start=(k == 0), stop=False,
                                    )

                # ---- fill: xcat row0 = observed ? y : pred
                with nc.named_scope("fill"):
                    if t > 0:
                        yfill = xio.tile([1, GP], F32, tag="yfill")
                        nc.vector.tensor_scalar(
                            out=yfill[:], in0=yo_ps[:],
                            scalar1=bout_s[0:1, 0:1], scalar2=None, op0=OP.add,
                        )
                        yrow = xio.tile([1, GP], F32, tag="yrow")
                        nc.sync.dma_start(out=yrow[:], in_=y0_d[t : t + 1, :])
                        mrow = xio.tile([1, GP], U8, tag="mrow")
                        nc.sync.dma_start(out=mrow[:], in_=mk_d[t : t + 1, :])
                        nc.vector.copy_predicated(yfill[:], mrow[:], yrow[:])
                        nc.vector.tensor_copy(xcat[KY : KY + 1, :], yfill[:])
                        pred_sb = xio.tile([1, GP], F32, tag="pred_sb")
                        nc.scalar.copy(pred_sb[:], yo_ps[:])
                        nc.sync.dma_start(out=out_d[t - 1 : t, :], in_=pred_sb[:])
                    else:
                        nc.gpsimd.dma_start(out=xcat[KY : KY + 1, :], in_=y0_d[0:1, :])

                # ---- x0 += wy * yt (K=2: yt row + zero-weighted x row), relu
                with nc.named_scope("x0"):
                    x0_sb = acts.tile([128, 2, GP], MM_DT, tag="x0")
                    for jb in range(2):
                        nc.tensor.matmul(
                            x0_ps[:, jb, 0:GP], wy2_s[KY : KY + 2, ts(jb, 128)],
                            xcat[KY : KY + 2, :], start=False, stop=True,
                        )
                    nc.scalar.activation(
                        out=x0_sb[:, 0, :], in_=x0_ps[:, 0, 0:GP],
                        func=AF.Relu, bias=bin_s[:, 0:1],
                    )
                    nc.vector.tensor_scalar(
                        out=x0_sb[:, 1, :], in0=x0_ps[:, 1, 0:GP],
                        scalar1=bin_s[:, 1:2], scalar2=0.0,
                        op0=OP.add, op1=OP.max,
                    )

                # ---- gate waves 0..2 (Wih) with cell c-chain interleaved
                c_new = state.tile([128, 2, GP], F32, tag="c")
                h_new = state.tile([128, 2, GP], MM_DT, tag="h")
                tc_t = acts.tile([128, 2, GP], F32, tag="tanh_c")
                tmp = acts.tile([128, 2, GP], F32, tag="tmp")
                gact = []
                for w in range(3):
                    with nc.named_scope(f"wave{w}"):
                        a_sb = acts.tile([128, 2, GP], F32, tag=f"act{w}")
                        for jb in range(2):
                            col = ts(2 * w + jb, 128)
                            for k in range(2):
                                nc.tensor.matmul(
                                    g_pss[w][jb][:, 0:GP], wih_s[:, k, col],
                                    x0_sb[:, k, :],
                                    start=(t == 0 and k == 0),
                                    stop=(k == 1),
                                )
                            nc.scalar.activation(
                                out=a_sb[:, jb, :], in_=g_pss[w][jb][:, 0:GP],
                                func=AF.Tanh if w == 1 else AF.Sigmoid,
                                bias=bg_s[:, 2 * w + jb : 2 * w + jb + 1],
                            )
                        gact.append(a_sb)
                    if w == 0 and t > 0:
                        # c_new = sigmoid(f) * c_prev as soon as f is ready
                        with nc.named_scope("cell"):
                            for jb in range(2):
                                nc.vector.tensor_mul(
                                    c_new[:, jb, :],
                                    gact[0][:, jb, :], c_prev[:, jb, :],
                                )
                    if w == 2:
                        # tmp = sigmoid(i)*tanh(g); c += tmp; tanh(c)
                        with nc.named_scope("cell"):
                            for jb in range(2):
                                nc.vector.tensor_mul(
                                    tmp[:, jb, :],
                                    gact[2][:, jb, :], gact[1][:, jb, :],
                                )
                                if t > 0:
                                    nc.vector.tensor_add(
                                        c_new[:, jb, :],
                                        c_new[:, jb, :], tmp[:, jb, :],
                                    )
                                else:
                                    nc.vector.tensor_copy(
                                        c_new[:, jb, :], tmp[:, jb, :]
                                    )
                                nc.scalar.activation(
                                    out=tc_t[:, jb, :], in_=c_new[:, jb, :],
                                    func=AF.Tanh,
                                )

                # ---- wave3 (o gate) + h per H-half
                with nc.named_scope("wave3"):
                    so = acts.tile([128, 2, GP], F32, tag="act3")
                    for jb in range(2):
                        g_ps = ps_g.tile([128, 512], F32, tag="gates",
                                         name=f"gps3{jb}")
                        col = ts(6 + jb, 128)
                        if t > 0:
                            for k in range(2):
                                nc.tensor.matmul(
                                    g_ps[:, 0:GP], whh_s[:, k, col],
                                    h_prev[:, k, :], start=(k == 0), stop=False,
                                )
                        for k in range(2):
                            nc.tensor.matmul(
                                g_ps[:, 0:GP], wih_s[:, k, col],
                                x0_sb[:, k, :],
                                start=(t == 0 and k == 0), stop=(k == 1),
                            )
                        nc.scalar.activation(
                            out=so[:, jb, :], in_=g_ps[:, 0:GP],
                            func=AF.Sigmoid,
                            bias=bg_s[:, 6 + jb : 6 + jb + 1],
                        )
                        nc.vector.tensor_mul(
                            h_new[:, jb, :], so[:, jb, :], tc_t[:, jb, :]
                        )

                with nc.named_scope("warm"):
                    dmy = ps_g.tile([128, 512], F32, tag="gates", name="dmy")
                    for d in range(5):
                        nc.tensor.matmul(
                            dmy[:, 0:GP], whh_s[:, 0, ts(d, 128)],
                            x0_sb[:, 0, :], start=True, stop=True,
                        )

                h_prev, c_prev = h_new, c_new

            # final output row from h_{NT-1}
            with nc.named_scope("pred"):
                yo_ps = ps_g.tile([1, GP], F32, tag="gates", name="yo_ps")
                nc.tensor.matmul(
                    yo_ps[:], wout_s[:, 0:1], h_prev[:, 0, :],
                    start=True, stop=False,
                )
                nc.tensor.matmul(
                    yo_ps[:], wout_s[:, 1:2], h_prev[:, 1, :],
                    start=False, stop=True,
                )
                pred_sb = xio.tile([1, GP], F32, tag="pred_sb")
                nc.scalar.copy(pred_sb[:], yo_ps[:])
                nc.sync.dma_start(out=out_d[NT - 1 : NT, :], in_=pred_sb[:])

    nc.finalize()
    return nc


def kernel(x, y, w_in, b_in, w_ih, b_ih, w_hh, b_hh, w_out, b_out):
    global LAST_EXEC_NS, LAST_RESULTS
    x = np.asarray(x, np.float32)
    y = np.asarray(y, np.float32)

    # gate reorder [i,f,g,o] -> wave order [f,g,i,o]
    perm = np.concatenate(
        [np.arange(H, 2 * H), np.arange(2 * H, 3 * H), np.arange(0, H),
         np.arange(3 * H, 4 * H)]
    )
    wih_r = np.asarray(w_ih, np.float32)[perm]          # [1024, 256]
    whh_r = np.asarray(w_hh, np.float32)[perm]
    bg_r = (np.asarray(b_ih, np.float32) + np.asarray(b_hh, np.float32))[perm]

    wih_dev = np.ascontiguousarray(
        wih_r.T.reshape(2, 128, 4 * H).transpose(1, 0, 2))  # [128,2,1024]
    whh_dev = np.ascontiguousarray(
        whh_r.T.reshape(2, 128, 4 * H).transpose(1, 0, 2))
    bg_dev = np.ascontiguousarray(bg_r.reshape(8, 128).T)   # [128,8]

    # winT covers xcat rows 1..22 (20 x features + 2 zero rows);
    # wy2T covers xcat rows 0..1 (y-feedback row + zero-weighted x row)
    w_in = np.asarray(w_in, np.float32)                      # [256, 21]
    win_re = np.concatenate(
        [w_in[:, :NX], np.zeros((H, 2), np.float32)], axis=1)  # [256, 22]
    win_dev = np.ascontiguousarray(win_re.T)                 # [22, 256]
    wy2_dev = np.ascontiguousarray(np.stack(
        [w_in[:, NX], np.zeros(H, np.float32)]))             # [2, 256]
    bin_dev = np.ascontiguousarray(
        np.asarray(b_in, np.float32).reshape(2, 128).T)      # [128,2]

    wout_dev = np.ascontiguousarray(
        np.asarray(w_out, np.float32).reshape(2, 128).T)     # [128,2]
    bout_dev = np.asarray(b_out, np.float32).reshape(1, 1)
    bout_f = float(np.asarray(b_out).reshape(-1)[0])

    y2 = y[:, :, 0]                                          # [NT, NGRID]
    mk_full = (~np.isnan(y2)).astype(np.uint8)               # 1 where observed
    y0_full = np.nan_to_num(y2, nan=0.0).astype(np.float32)

    if MM_DT == mybir.dt.bfloat16:
        import ml_dtypes
        cast = lambda a: np.asarray(a).astype(ml_dtypes.bfloat16)
    else:
        cast = lambda a: a
    wih_dev, whh_dev, win_dev, wout_dev, wy2_dev = map(
        cast, (wih_dev, whh_dev, win_dev, wout_dev, wy2_dev))
    zz = cast(np.zeros((1, GP), np.float32))
    nc = build_nc()
    in_maps = []
    for c in range(NCORES):
        g0, g1 = c * G, (c + 1) * G
        xT = np.zeros((NT, KXF, GP), np.float32)
        xT[:, :NX, :G] = x[:, g0:g1, :].transpose(0, 2, 1)
        xT = cast(xT)
        y0 = np.zeros((NT, GP), np.float32)
        y0[:, :G] = y0_full[:, g0:g1]
        mk = np.zeros((NT, GP), np.uint8)
        mk[:, :G] = mk_full[:, g0:g1]
        in_maps.append(
            {
                "xT": xT, "y0": y0, "mk": mk, "zz": zz,
                "wihT": wih_dev, "whhT": whh_dev, "winT": win_dev, "wy2T": wy2_dev,
                "woutT": wout_dev, "bg": bg_dev, "bin": bin_dev,
                "bout": bout_dev,
            }
        )

    res = None
    for attempt in range(3):
        try:
            res = run_bass_kernel_spmd(nc, in_maps, core_ids=list(range(NCORES)))
            break
        except Exception:
            if attempt == 2:
                raise
    LAST_EXEC_NS = res.exec_time_ns
    LAST_RESULTS = res

    out = np.empty((NT, NGRID, NY), np.float32)
    for c in range(NCORES):
        out[:, c * G : (c + 1) * G, 0] = res.results[c]["outy"][:, :G] + bout_f
    return out

